# revision 1
# baseline (speedup 1.0000x reference)
"""Causal self-attention (B=4, S=2048, D=1024, single head) on 8 trn2 cores.

Sharding: data-parallel over batch (4 batches x 2 cores). The two cores of a
batch split the 8 query tiles of 256 rows by parity (core even: tiles
{0,2,4,6}, odd: {1,3,5,7}) so every core runs the *same* block schedule
(uniform SPMD program); causality and tile position enter only through
per-core input data (query-row gather + additive mask tiles).

Per-core kernel: project K^T / V / Q^T with fp32r matmuls (X and W transposed
on the tensor engine against an identity), stage V through a DRAM scratch to
fit SBUF, then blocked attention: scoresT[k,q] blocks accumulate in PSUM,
additive causal mask (DVE), exp via ScalarE (scale=1/sqrt(D)) straight into
fp32r SBUF tiles that feed the AV matmuls; row-sums ride along as an extra
N=2 matmul against a ones vector sharing the AV lhsT; normalize fuses into
the PSUM->SBUF eviction as a per-partition scalar multiply.
"""

import os
import numpy as np
from contextlib import ExitStack

import concourse.bass as bass
import concourse.tile as tile
import concourse.mybir as mybir
from concourse import bacc
from concourse.bass_utils import run_bass_kernel_spmd
from concourse.masks import make_identity

F32 = mybir.dt.float32
F32R = mybir.dt.float32r
AFT = mybir.ActivationFunctionType

B, S, D = 4, 2048, 1024
P = 128
QTILE = 256          # queries per attention tile (4 tiles/core)
NT = 4               # attention tiles per core
SB = 256             # rows per projection s-block
NSB = S // SB        # 8
DC = D // P          # 8 contraction chunks
SCALE = 1.0 / np.sqrt(D)
MASK_NEG = -1.0e9

N_KC = [4 * (t + 1) for t in range(NT)]          # kc blocks per tile: 4,8,12,16
NBLK = sum(N_KC)                                  # 40
BLK0 = [sum(N_KC[:t]) for t in range(NT)]         # block offsets per tile

_NC_CACHE = None
_PHASES = os.environ.get("KPHASES", "BCD")  # which phases to emit (sim attribution)


def _build():
    nc = bacc.Bacc("TRN2", target_bir_lowering=False, debug=False, num_devices=8)
    xq = nc.dram_tensor("Xq", [NT * QTILE, D], F32, kind="ExternalInput").ap()
    xkv = nc.dram_tensor("Xkv", [S, D], F32, kind="ExternalInput").ap()
    wq = nc.dram_tensor("Wq", [D, D], F32, kind="ExternalInput").ap()
    wk = nc.dram_tensor("Wk", [D, D], F32, kind="ExternalInput").ap()
    wv = nc.dram_tensor("Wv", [D, D], F32, kind="ExternalInput").ap()
    msk = nc.dram_tensor("Mask", [NT, 4, P, QTILE], F32, kind="ExternalInput").ap()
    out = nc.dram_tensor("O", [NT * QTILE, D], F32, kind="ExternalOutput").ap()

    HK = S // (2 * P)   # 8 kc chunks per K/V half

    with tile.TileContext(nc) as tc, ExitStack() as ctx:
        big = ctx.enter_context(tc.tile_pool(name="big", bufs=1))
        dram = ctx.enter_context(tc.tile_pool(name="dram", bufs=1, space="DRAM"))

        ident = big.tile([P, P], F32)
        make_identity(nc, ident[:])
        ones_f = big.tile([P, 2], F32)
        nc.vector.memset(ones_f[:], 1.0)
        ones2 = big.tile([P, 2], F32R)
        nc.vector.tensor_copy(ones2[:], ones_f[:])

        # resident low halves (k < 1024); high halves staged via DRAM
        KTlo = big.tile([P, DC, S // 2], F32R)     # [e-part, ec, k]
        Vlo = big.tile([P, HK, D], F32R)           # [k-part, kc, e]
        KTdram = dram.tile([DC, P, S // 2], F32R)
        Vdram = dram.tile([S // 2, D], F32R)

        tb_ctr = [0]

        def transpose_block(psum, src_ap, dst_ap):
            """PE-transpose a [128,128] f32 block; evict psum->sbuf rounds to f32r.
            Evictions go 3:1 DVE:ACT."""
            pt = psum.tile([P, P], F32, tag="tp")
            nc.tensor.transpose(pt[:], src_ap, ident[:])
            tb_ctr[0] += 1
            if tb_ctr[0] % 4 == 0:
                nc.scalar.copy(dst_ap, pt[:])
            else:
                nc.vector.tensor_copy(dst_ap, pt[:])

        def load_wT(wpool, spool, psum, wap, name):
            """Load W [e,d] natural, PE-transpose into W^T [d-part, dc, e] fp32r."""
            wt = wpool.tile([P, DC, D], F32R, tag=f"wt_{name}")
            for ec in range(DC):
                wn = spool.tile([P, D], F32, tag="wnat")
                nc.sync.dma_start(wn[:], wap[ec * P:(ec + 1) * P, :])
                for dc in range(DC):
                    transpose_block(psum, wn[:, dc * P:(dc + 1) * P],
                                    wt[:, dc, ec * P:(ec + 1) * P])
            return wt

        # ---------------- Phase B: K^T and V ----------------
        if "B" in _PHASES:
         with tc.tile_pool(name="projB_w", bufs=1) as pbw, \
             tc.tile_pool(name="projB", bufs=2) as pb, \
             tc.tile_pool(name="projB_x", bufs=1) as pbx, \
             tc.tile_pool(name="projB_ps", bufs=2, space="PSUM") as pbps:
            # hi s-blocks (DRAM-staged) first so phase B's tail has no pending
            # stores; lo s-blocks (SBUF-resident evictions) last. Prefetch +
            # transpose the first s-block of X before the 8MB of W loads.
            SB_ORDER = [4, 5, 6, 7, 0, 1, 2, 3]
            xt0 = pbx.tile([P, DC, SB], F32R, tag="xt")
            for i in range(SB // P):
                xn = pb.tile([P, D], F32, tag="xn")
                r = SB_ORDER[0] * (SB // P) + i
                nc.sync.dma_start(xn[:], xkv[r * P:(r + 1) * P, :])
                for dc in range(DC):
                    transpose_block(pbps, xn[:, dc * P:(dc + 1) * P],
                                    xt0[:, dc, i * P:(i + 1) * P])
            wkt = load_wT(pbw, pb, pbps, wk, "k")
            wvt = load_wT(pbw, pb, pbps, wv, "v")
            for si, sb in enumerate(SB_ORDER):
                lo = sb < NSB // 2
                if si == 0:
                    xt = xt0
                else:
                    xt = pbx.tile([P, DC, SB], F32R, tag="xt")
                    for i in range(SB // P):
                        xn = pb.tile([P, D], F32, tag="xn")
                        nc.sync.dma_start(xn[:], xkv[(sb * (SB // P) + i) * P:(sb * (SB // P) + i + 1) * P, :])
                        for dc in range(DC):
                            transpose_block(pbps, xn[:, dc * P:(dc + 1) * P],
                                            xt[:, dc, i * P:(i + 1) * P])
                # K^T columns for this s-block
                if lo:
                    for ec in range(DC):
                        pk = pbps.tile([P, SB], F32, tag="pk")
                        for dc in range(DC):
                            nc.tensor.matmul(pk[:], wkt[:, dc, ec * P:(ec + 1) * P],
                                             xt[:, dc, :], start=(dc == 0), stop=(dc == DC - 1))
                        nc.scalar.copy(KTlo[:, ec, sb * SB:(sb + 1) * SB], pk[:])
                else:
                    kst = pb.tile([P, DC, SB], F32R, tag="kst")
                    for ec in range(DC):
                        pk = pbps.tile([P, SB], F32, tag="pk")
                        for dc in range(DC):
                            nc.tensor.matmul(pk[:], wkt[:, dc, ec * P:(ec + 1) * P],
                                             xt[:, dc, :], start=(dc == 0), stop=(dc == DC - 1))
                        nc.scalar.copy(kst[:, ec, :], pk[:])
                    nc.sync.dma_start(
                        KTdram[:, :, (sb - NSB // 2) * SB:(sb - NSB // 2 + 1) * SB].rearrange("ec p k -> p ec k"),
                        kst[:])
                # V rows for this s-block
                if lo:
                    for i in range(SB // P):
                        for e2 in range(2):
                            pv = pbps.tile([P, 512], F32, tag="pv")
                            for dc in range(DC):
                                nc.tensor.matmul(pv[:], xt[:, dc, i * P:(i + 1) * P],
                                                 wvt[:, dc, e2 * 512:(e2 + 1) * 512],
                                                 start=(dc == 0), stop=(dc == DC - 1))
                            nc.scalar.copy(Vlo[:, sb * (SB // P) + i, e2 * 512:(e2 + 1) * 512], pv[:])
                else:
                    vst = pb.tile([P, SB // P, D], F32R, tag="vst")
                    for i in range(SB // P):
                        for e2 in range(2):
                            pv = pbps.tile([P, 512], F32, tag="pv")
                            for dc in range(DC):
                                nc.tensor.matmul(pv[:], xt[:, dc, i * P:(i + 1) * P],
                                                 wvt[:, dc, e2 * 512:(e2 + 1) * 512],
                                                 start=(dc == 0), stop=(dc == DC - 1))
                            nc.scalar.copy(vst[:, i, e2 * 512:(e2 + 1) * 512], pv[:])
                    nc.sync.dma_start(
                        Vdram[(sb - NSB // 2) * SB:(sb - NSB // 2 + 1) * SB, :].rearrange("(i p) e -> p i e", p=P),
                        vst[:])

        # ---------------- Phases C+D persistent ----------------
        persist2 = ctx.enter_context(tc.tile_pool(name="persist2", bufs=1))
        QT = persist2.tile([P, DC, NT * QTILE], F32R)  # Q^T [e-part, ec, q]

        # ---------------- Phase C: Q^T ----------------
        if "C" in _PHASES:
         with tc.tile_pool(name="projC_w", bufs=1) as pcw, \
             tc.tile_pool(name="projC", bufs=4) as pc, \
             tc.tile_pool(name="projC_ps", bufs=2, space="PSUM") as pcps:
            xtq = pcw.tile([P, DC, NT * QTILE], F32R, tag="xtq")
            for i in range(NT * QTILE // P):
                xn = pc.tile([P, D], F32, tag="xn")
                nc.sync.dma_start(xn[:], xq[i * P:(i + 1) * P, :])
                for dc in range(DC):
                    transpose_block(pcps, xn[:, dc * P:(dc + 1) * P],
                                    xtq[:, dc, i * P:(i + 1) * P])
            wqt = load_wT(pcw, pc, pcps, wq, "q")
            for ec in range(DC):
                for qc in range(NT * QTILE // 512):
                    pq = pcps.tile([P, 512], F32, tag="pq")
                    for dc in range(DC):
                        nc.tensor.matmul(pq[:], wqt[:, dc, ec * P:(ec + 1) * P],
                                         xtq[:, dc, qc * 512:(qc + 1) * 512],
                                         start=(dc == 0), stop=(dc == DC - 1))
                    nc.scalar.copy(QT[:, ec, qc * 512:(qc + 1) * 512], pq[:])

        # ---------------- Phase D: attention ----------------
        if "D" in _PHASES:
         with tc.tile_pool(name="attn", bufs=2) as pa, \
             tc.tile_pool(name="attn_e", bufs=1) as pe_pool, \
             tc.tile_pool(name="attn_m", bufs=2) as pm, \
             tc.tile_pool(name="attn_o", bufs=1) as po, \
             tc.tile_pool(name="hi", bufs=1) as phi, \
             tc.tile_pool(name="attn_s", bufs=3, space="PSUM") as psS, \
             tc.tile_pool(name="attn_u", bufs=2, space="PSUM") as psU, \
             tc.tile_pool(name="attn_r", bufs=1, space="PSUM") as psR:
            KThi = phi.tile([P, DC, S // 2], F32R)
            Vhi = phi.tile([P, HK, D], F32R)
            # tile-0 mask first so it isn't queued behind the 8MB hi loads
            mtiles = []
            m0 = pm.tile([P, 4, QTILE], F32, tag="mtile")
            nc.gpsimd.dma_start(m0[:], msk[0].rearrange("b p j -> p b j"))
            mtiles.append(m0)
            # split hi reloads: tile 2 needs only kc 8..11, tile 3 the rest
            nc.sync.dma_start(KThi[:, :, 0:512],
                              KTdram[:, :, 0:512].rearrange("ec p k -> p ec k"))
            nc.sync.dma_start(Vhi[:, 0:4, :],
                              Vdram[0:512, :].rearrange("(kc p) e -> p kc e", p=P))
            nc.sync.dma_start(KThi[:, :, 512:1024],
                              KTdram[:, :, 512:1024].rearrange("ec p k -> p ec k"))
            nc.sync.dma_start(Vhi[:, 4:8, :],
                              Vdram[512:1024, :].rearrange("(kc p) e -> p kc e", p=P))

            def KTat(ec, kc):
                if kc < HK:
                    return KTlo[:, ec, kc * P:(kc + 1) * P]
                return KThi[:, ec, (kc - HK) * P:(kc - HK + 1) * P]

            def Vat(kc, esl):
                if kc < HK:
                    return Vlo[:, kc, esl]
                return Vhi[:, kc - HK, esl]

            for t in range(NT):
                n = N_KC[t]
                mtile = mtiles[t]
                if t + 1 < NT:  # prefetch next tile's mask blocks
                    mnext = pm.tile([P, 4, QTILE], F32, tag="mtile")
                    nc.gpsimd.dma_start(mnext[:], msk[t + 1].rearrange("b p j -> p b j"))
                    mtiles.append(mnext)
                expS = pe_pool.tile([P, 16, QTILE], F32R, tag="expS")
                for kc in range(n):
                    pS = psS.tile([P, QTILE], F32, tag="pS")
                    for ec in range(DC):
                        nc.tensor.matmul(pS[:], KTat(ec, kc),
                                         QT[:, ec, t * QTILE:(t + 1) * QTILE],
                                         start=(ec == 0), stop=(ec == DC - 1))
                    if kc >= n - 4:  # only the 4 diagonal-edge blocks carry a mask
                        nc.vector.tensor_add(pS[:], pS[:], mtile[:, kc - (n - 4), :])
                    nc.scalar.activation(expS[:, kc, :], pS[:], AFT.Exp, scale=SCALE)
                for qc in range(QTILE // P):
                    pU0 = psU.tile([P, 512], F32, tag="pU0")
                    pU1 = psU.tile([P, 512], F32, tag="pU1")
                    pR = psR.tile([P, 2], F32, tag="pR")
                    for kc in range(n):
                        lhs = expS[:, kc, qc * P:(qc + 1) * P]
                        st, sp = (kc == 0), (kc == n - 1)
                        nc.tensor.matmul(pU0[:], lhs, Vat(kc, slice(0, 512)), start=st, stop=sp)
                        nc.tensor.matmul(pU1[:], lhs, Vat(kc, slice(512, 1024)), start=st, stop=sp)
                        nc.tensor.matmul(pR[:], lhs, ones2[:], start=st, stop=sp)
                    rsb = pa.tile([P, 1], F32, tag="rsb")
                    recip = pa.tile([P, 1], F32, tag="recip")
                    nc.vector.tensor_copy(rsb[:], pR[:, 0:1])
                    nc.vector.reciprocal(recip[:], rsb[:])
                    ot = po.tile([P, D], F32, tag="ot")
                    nc.vector.tensor_scalar_mul(ot[:, 0:512], pU0[:], recip[:])
                    nc.vector.tensor_scalar_mul(ot[:, 512:1024], pU1[:], recip[:])
                    nc.sync.dma_start(out[(t * QTILE + qc * P):(t * QTILE + (qc + 1) * P), :], ot[:])

    nc.compile()
    return nc


def _get_nc():
    global _NC_CACHE
    if _NC_CACHE is None:
        _NC_CACHE = _build()
    return _NC_CACHE


def _make_masks(parity: int) -> np.ndarray:
    """Masks for the last 4 kc blocks of each tile (earlier blocks are fully
    visible for both parities)."""
    m = np.empty((NT, 4, P, QTILE), dtype=np.float32)
    j = np.arange(QTILE)[None, :]
    p = np.arange(P)[:, None]
    for t in range(NT):
        g = 2 * t + parity
        n = N_KC[t]
        for s in range(4):
            kc = n - 4 + s
            qglob = g * QTILE + j
            kglob = kc * P + p
            m[t, s] = np.where(qglob >= kglob, 0.0, MASK_NEG)
    return m


def kernel(X, W_q, W_k, W_v):
    X = np.asarray(X, dtype=np.float32)
    W_q = np.asarray(W_q, dtype=np.float32)
    W_k = np.asarray(W_k, dtype=np.float32)
    W_v = np.asarray(W_v, dtype=np.float32)

    masks = [_make_masks(par) for par in range(2)]
    in_maps = []
    for c in range(8):
        b, par = c // 2, c % 2
        rows = np.concatenate([X[b, (2 * t + par) * QTILE:(2 * t + par + 1) * QTILE, :]
                               for t in range(NT)], axis=0)
        in_maps.append({
            "Xq": np.ascontiguousarray(rows),
            "Xkv": np.ascontiguousarray(X[b]),
            "Wq": W_q, "Wk": W_k, "Wv": W_v,
            "Mask": masks[par],
        })

    global _last_in_maps
    _last_in_maps = in_maps
    nc = _get_nc()
    res = run_bass_kernel_spmd(nc, in_maps, core_ids=list(range(8)))

    out = np.empty((B, S, D), dtype=np.float32)
    for c in range(8):
        b, par = c // 2, c % 2
        oc = res.results[c]["O"]
        for t in range(NT):
            g = 2 * t + par
            out[b, g * QTILE:(g + 1) * QTILE, :] = oc[t * QTILE:(t + 1) * QTILE, :]
    return out



# revision 2
# speedup vs baseline: 1.7946x; 1.7946x over previous
"""Causal self-attention (B=4, S=2048, D=1024) on 8 trn2 cores, v4.

Sharding: the two cores of a batch split the KEY/VALUE space by interleaved
128-row blocks (even core owns kc = 0,2,4,..., odd kc = 1,3,5,...). Each
core projects K^T/V only for its own 8 blocks, projects Q^T for ALL 2048
queries of its batch, and runs attention of every query tile against its
own key blocks: tile g (256 queries) sees exactly g+1 own-blocks on either
core, so the block schedule is uniform SPMD; parity enters only through
input data (gathered X^T columns + one diagonal mask per tile). Cores
output the unnormalized AV partials U and the exp row-sums r; the host
merges the pair: out = (U_even + U_odd) / (r_even + r_odd).

This removes the pairwise K/V projection redundancy without collectives:
per-core PE work is ~414k cycles (~173us @2.4GHz) vs ~497k in v2.
Everything is bf16 (host-side transposes/casts), fully SBUF-resident,
batched single-descriptor DMA loads, stationary-operand-reuse loop orders,
and score tiles are processed in pairs so the shared kc prefix runs at
moving-dim 512.
"""

import numpy as np
from contextlib import ExitStack

import concourse.bass as bass
import concourse.tile as tile
import concourse.mybir as mybir
from concourse import bacc
from concourse.bass_utils import run_bass_kernel_spmd

F32 = mybir.dt.float32
BF16 = mybir.dt.bfloat16
AFT = mybir.ActivationFunctionType
NP_BF16 = mybir.dt.np(mybir.dt.bfloat16)

B, S, D = 4, 2048, 1024
P = 128
QTILE = 256
NG = S // QTILE      # 8 query tiles per core (all of the batch)
DC = D // P          # 8
EC = D // P          # 8
NKO = 8              # own kc blocks per core
SB = 512
SCALE = 1.0 / np.sqrt(D)
MASK_NEG = -1.0e9

_NC_CACHE = None


def _emit(nc, tc, ctx, xt, xkv, wqt, wkt, wvt, msk, out, rout):
    persist = ctx.enter_context(tc.tile_pool(name="persist", bufs=1))

    ones2 = persist.tile([P, 2], BF16)
    nc.vector.memset(ones2[:], 1.0)

    KT = persist.tile([P, EC, NKO * P], BF16)   # K^T own: [e-part, ec, kslot*128]
    V = persist.tile([P, NKO, D], BF16)         # V own:   [k-part, kslot, e]
    QT = persist.tile([P, EC, S], BF16)         # Q^T all: [e-part, ec, q]
    mt = persist.tile([P, NG, QTILE], F32)      # one diagonal mask per tile

    # ---------------- loads + projections ----------------
    with tc.tile_pool(name="proj", bufs=1) as proj:
        xts = proj.tile([P, DC, S], BF16)       # X^T full (queries = all cols)
        xkvs = proj.tile([P, DC, NKO * P], BF16)  # X^T own kc cols (gathered)
        wk = proj.tile([P, DC, D], BF16)
        wv = proj.tile([P, DC, D], BF16)
        wq = proj.tile([P, DC, D], BF16)

        nc.sync.dma_start(wk[:, :, 0:512],
                          wkt[:, 0:512].rearrange("(dc p) e -> p dc e", p=P))
        nc.gpsimd.dma_start(xkvs[:],
                            xkv.rearrange("(dc p) k -> p dc k", p=P))
        nc.sync.dma_start(wk[:, :, 512:1024],
                          wkt[:, 512:1024].rearrange("(dc p) e -> p dc e", p=P))
        nc.sync.dma_start(wv[:], wvt.rearrange("(dc p) e -> p dc e", p=P))
        nc.gpsimd.dma_start(xts[:, :, 0:S // 2],
                            xt[:, 0:S // 2].rearrange("(dc p) s -> p dc s", p=P))
        nc.sync.dma_start(wq[:], wqt.rearrange("(dc p) e -> p dc e", p=P))
        nc.gpsimd.dma_start(xts[:, :, S // 2:S],
                            xt[:, S // 2:S].rearrange("(dc p) s -> p dc s", p=P))
        nc.gpsimd.dma_start(mt[:], msk.rearrange("g p j -> p g j"))

        # ---- K^T own: [e, 1024] ----
        with tc.tile_pool(name="kproj_ps", bufs=3, space="PSUM") as kps:
            for ec in range(EC):
                pss = [kps.tile([P, SB], F32, tag=f"pk{sb}", name=f"pk{sb}")
                       for sb in range(2)]
                for dc in range(DC):
                    for sb in range(2):
                        nc.tensor.matmul(pss[sb][:], wk[:, dc, ec * P:(ec + 1) * P],
                                         xkvs[:, dc, sb * SB:(sb + 1) * SB],
                                         start=(dc == 0), stop=(dc == DC - 1))
                for sb in range(2):
                    if (ec + sb) % 2 == 0:
                        nc.scalar.copy(KT[:, ec, sb * SB:(sb + 1) * SB], pss[sb][:])
                    else:
                        nc.vector.tensor_copy(KT[:, ec, sb * SB:(sb + 1) * SB], pss[sb][:])

        # ---- V own: [1024, e] ----
        with tc.tile_pool(name="vproj_ps", bufs=3, space="PSUM") as vps:
            for kc in range(NKO):
                pss = [vps.tile([P, 512], F32, tag=f"pv{eh}", name=f"pv{eh}")
                       for eh in range(2)]
                for dc in range(DC):
                    for eh in range(2):
                        nc.tensor.matmul(pss[eh][:], xkvs[:, dc, kc * P:(kc + 1) * P],
                                         wv[:, dc, eh * 512:(eh + 1) * 512],
                                         start=(dc == 0), stop=(dc == DC - 1))
                nc.scalar.copy(V[:, kc, 0:512], pss[0][:])
                nc.vector.tensor_copy(V[:, kc, 512:1024], pss[1][:])

        # ---- Q^T all queries: [e, 2048] ----
        with tc.tile_pool(name="qproj_ps", bufs=2, space="PSUM") as qps:
            for ec in range(EC):
                pss = [qps.tile([P, 512], F32, tag=f"pq{qh}", name=f"pq{qh}")
                       for qh in range(4)]
                for dc in range(DC):
                    for qh in range(4):
                        nc.tensor.matmul(pss[qh][:], wq[:, dc, ec * P:(ec + 1) * P],
                                         xts[:, dc, qh * 512:(qh + 1) * 512],
                                         start=(dc == 0), stop=(dc == DC - 1))
                for qh in range(4):
                    if qh % 2 == 0:
                        nc.scalar.copy(QT[:, ec, qh * 512:(qh + 1) * 512], pss[qh][:])
                    else:
                        nc.vector.tensor_copy(QT[:, ec, qh * 512:(qh + 1) * 512], pss[qh][:])

    # ---------------- attention ----------------
    # tiles processed in pairs (g, g+1): the shared kc prefix (slots 0..g)
    # runs as [128, 512] score blocks covering both tiles' queries; tile
    # g+1's extra slot g+1 runs as a [128, 256] tail block.
    with tc.tile_pool(name="attn_e", bufs=2) as pe_pool, \
         tc.tile_pool(name="attn", bufs=2) as pa, \
         tc.tile_pool(name="attn_o", bufs=4) as po, \
         tc.tile_pool(name="attn_s", bufs=3, space="PSUM") as psS, \
         tc.tile_pool(name="attn_u", bufs=2, space="PSUM") as psU, \
         tc.tile_pool(name="attn_r", bufs=1, space="PSUM") as psR:
        rt = pa.tile([P, 2 * NG], F32, tag="rt")   # row-sum slots [(g,qc)]
        for gp in range(NG // 2):
            g0 = 2 * gp                # tiles g0 (left q-half) and g0+1 (right)
            expS = pe_pool.tile([P, NKO, 2 * QTILE], BF16, tag="expS")
            # prefix slots 0..g0 at N=512 over both tiles' queries
            for j in range(g0 + 1):
                pS = psS.tile([P, 2 * QTILE], F32, tag="pS")
                for ec in range(EC):
                    nc.tensor.matmul(pS[:], KT[:, ec, j * P:(j + 1) * P],
                                     QT[:, ec, g0 * QTILE:(g0 + 2) * QTILE],
                                     start=(ec == 0), stop=(ec == EC - 1))
                if j == g0:  # diagonal of tile g0 (left half); right half visible
                    nc.vector.tensor_add(pS[:, 0:QTILE], pS[:, 0:QTILE], mt[:, g0, :])
                nc.scalar.activation(expS[:, j, :], pS[:], AFT.Exp, scale=SCALE)
            # tail slot g0+1 for tile g0+1 only (right q-half)
            pSt = psS.tile([P, 2 * QTILE], F32, tag="pS")
            for ec in range(EC):
                nc.tensor.matmul(pSt[:, 0:QTILE], KT[:, ec, (g0 + 1) * P:(g0 + 2) * P],
                                 QT[:, ec, (g0 + 1) * QTILE:(g0 + 2) * QTILE],
                                 start=(ec == 0), stop=(ec == EC - 1))
            nc.vector.tensor_add(pSt[:, 0:QTILE], pSt[:, 0:QTILE], mt[:, g0 + 1, :])
            nc.scalar.activation(expS[:, g0 + 1, 256:512], pSt[:, 0:QTILE], AFT.Exp, scale=SCALE)

            # AV + row-sums for both tiles of the pair
            for half in range(2):
                g = g0 + half
                nsl = g + 1
                for qc in range(QTILE // P):
                    pU0 = psU.tile([P, 512], F32, tag="pU0")
                    pU1 = psU.tile([P, 512], F32, tag="pU1")
                    pR = psR.tile([P, 2], F32, tag="pR")
                    for j in range(nsl):
                        lhs = expS[:, j, half * QTILE + qc * P: half * QTILE + (qc + 1) * P]
                        st, sp = (j == 0), (j == nsl - 1)
                        nc.tensor.matmul(pU0[:], lhs, V[:, j, 0:512], start=st, stop=sp)
                        nc.tensor.matmul(pU1[:], lhs, V[:, j, 512:1024], start=st, stop=sp)
                        nc.tensor.matmul(pR[:], lhs, ones2[:], start=st, stop=sp)
                    nc.vector.tensor_copy(rt[:, 2 * g + qc: 2 * g + qc + 1], pR[:, 0:1])
                    ot = po.tile([P, D], F32, tag="ot")
                    nc.scalar.copy(ot[:, 0:512], pU0[:])
                    nc.vector.tensor_copy(ot[:, 512:1024], pU1[:])
                    nc.sync.dma_start(out[(g * QTILE + qc * P):(g * QTILE + (qc + 1) * P), :], ot[:])
        nc.sync.dma_start(rout.rearrange("s p -> p s"), rt[:])


def _build(reps: int = 1):
    nc = bacc.Bacc("TRN2", target_bir_lowering=False, debug=False, num_devices=8)
    xt = nc.dram_tensor("XT", [D, S], BF16, kind="ExternalInput").ap()
    xkv = nc.dram_tensor("XkvT", [D, NKO * P], BF16, kind="ExternalInput").ap()
    wqt = nc.dram_tensor("WqT", [D, D], BF16, kind="ExternalInput").ap()
    wkt = nc.dram_tensor("WkT", [D, D], BF16, kind="ExternalInput").ap()
    wvt = nc.dram_tensor("WvT", [D, D], BF16, kind="ExternalInput").ap()
    msk = nc.dram_tensor("Mask", [NG, P, QTILE], F32, kind="ExternalInput").ap()
    out = nc.dram_tensor("O", [S, D], F32, kind="ExternalOutput").ap()
    rout = nc.dram_tensor("R", [2 * NG, P], F32, kind="ExternalOutput").ap()

    with tile.TileContext(nc) as tc:
        for _rep in range(reps):
            with ExitStack() as ctx:
                _emit(nc, tc, ctx, xt, xkv, wqt, wkt, wvt, msk, out, rout)

    nc.compile()
    return nc


def _get_nc():
    global _NC_CACHE
    if _NC_CACHE is None:
        _NC_CACHE = _build()
    return _NC_CACHE


def _make_masks(parity: int) -> np.ndarray:
    """One mask per tile g: for slot j=g (global kc = 2g+parity),
    zero where q >= k else -1e9."""
    m = np.empty((NG, P, QTILE), dtype=np.float32)
    j = np.arange(QTILE)[None, :]
    p = np.arange(P)[:, None]
    for g in range(NG):
        kglob = (2 * g + parity) * P + p
        qglob = g * QTILE + j
        m[g] = np.where(qglob >= kglob, 0.0, MASK_NEG)
    return m


def _prep_in_maps(X, W_q, W_k, W_v):
    X = np.asarray(X, dtype=np.float32)
    WqT = np.ascontiguousarray(np.asarray(W_q, np.float32).astype(NP_BF16).T)
    WkT = np.ascontiguousarray(np.asarray(W_k, np.float32).astype(NP_BF16).T)
    WvT = np.ascontiguousarray(np.asarray(W_v, np.float32).astype(NP_BF16).T)
    Xb16 = X.astype(NP_BF16)

    masks = [_make_masks(par) for par in range(2)]
    in_maps = []
    for c in range(8):
        b, par = c // 2, c % 2
        XTb = np.ascontiguousarray(Xb16[b].T)                    # [D, S]
        kcols = np.concatenate(
            [XTb[:, (2 * j + par) * P:(2 * j + par + 1) * P]
             for j in range(NKO)], axis=1)
        in_maps.append({
            "XT": XTb,
            "XkvT": np.ascontiguousarray(kcols),
            "WqT": WqT, "WkT": WkT, "WvT": WvT,
            "Mask": masks[par],
        })
    return in_maps


def kernel(X, W_q, W_k, W_v):
    in_maps = _prep_in_maps(X, W_q, W_k, W_v)
    global _last_in_maps
    _last_in_maps = in_maps
    nc = _get_nc()
    res = run_bass_kernel_spmd(nc, in_maps, core_ids=list(range(8)))

    out = np.empty((B, S, D), dtype=np.float32)
    for b in range(B):
        U0 = res.results[2 * b]["O"]
        U1 = res.results[2 * b + 1]["O"]
        r0 = res.results[2 * b]["R"].reshape(S)
        r1 = res.results[2 * b + 1]["R"].reshape(S)
        out[b] = (U0 + U1) / (r0 + r1)[:, None]
    return out
